# revision 8
# baseline (speedup 1.0000x reference)
"""Trainium2 Bass kernel computing out = x * exp(diagonal).

x: (8192, 4096) float32, diagonal: (4096,) float32.
Data-parallel across 8 NeuronCores: each core handles 1024 rows of x;
the diagonal is replicated to every core.

Per-core program (pure streaming; the 16 SDMA engines aggregate
~435 GB/s and bound the kernel). Two HW quirks shape the layout:

  * SDMA engine 15 runs ~15-20% slower than its 15 peers and serves the
    fixed partition set {92-95, 124-127}. With all 128 partitions in
    use it accumulates a 12-17 us backlog that delays every DMA
    completion semaphore and sets the kernel tail. The x tiles
    therefore use only partitions 0..119: engine 15 serves 4 instead
    of 8 partitions (half share) and never lags, while the other
    engines take +6.7% each.
  * A stride-0 DRAM broadcast of the 16 KiB diagonal re-reads one HBM
    page 128x and is single-channel-bound (~25 us), so the host
    replicates the diagonal to (120, 4096) and the partition broadcast
    becomes a plain full-rate ~1.9 MiB load.

Program: diagonal replica loads first on the SP queue, ACT exps it in
place (a 1-element DVE copy observes the Exp so later muls carry
exactly one wait). x streams through 4 fresh [120, 2, 4096] tiles
(row-folds consecutive in DRAM -> 32 KiB descriptors) plus one
[64, 1, 4096] remainder: HWDGE load on SP -> in-place DVE multiply per
row-fold -> HWDGE store on ACT.
"""

import numpy as np

BATCH, FEAT = 8192, 4096
N_CORES = 8
ROWS = BATCH // N_CORES   # 1024 rows per core
P = 120                   # partitions used by x tiles (engine 15 on half duty)
FOLD = 2                  # consecutive DRAM rows folded into one partition
N_TILES = 4               # 4 tiles of [120, 2, 4096] = 960 rows
REM = ROWS - N_TILES * P * FOLD  # 64 remainder rows as [64, 1, 4096]

_CACHE = {}


def build_nc(rows=ROWS, feat=FEAT, fold=FOLD):
    import concourse.bacc as bacc
    import concourse.mybir as mybir
    from concourse import tile

    # Bacc (not plain Bass): its compile() pass splits multi-sem waits into
    # EventSemaphore chains -- TRN2 instructions carry at most one wait.
    nc = bacc.Bacc("TRN2", target_bir_lowering=False, debug=False)
    x = nc.dram_tensor("x", (rows, feat), mybir.dt.float32, kind="ExternalInput").ap()
    d = nc.dram_tensor("d", (P, feat), mybir.dt.float32, kind="ExternalInput").ap()
    out = nc.dram_tensor(
        "out", (rows, feat), mybir.dt.float32, kind="ExternalOutput"
    ).ap()

    main_rows = N_TILES * P * fold
    # n INSIDE p: partition p holds `fold` consecutive DRAM rows -> one
    # 32 KiB contiguous descriptor per partition per DMA.
    x_t = x[0:main_rows].rearrange("(s p n) m -> s p n m", p=P, n=fold)
    o_t = out[0:main_rows].rearrange("(s p n) m -> s p n m", p=P, n=fold)
    x_r = x[main_rows:rows].rearrange("(p n) m -> p n m", n=1)
    o_r = out[main_rows:rows].rearrange("(p n) m -> p n m", n=1)

    with tile.TileContext(nc) as tc:
        with (
            tc.tile_pool(name="const", bufs=1) as cpool,
            tc.tile_pool(name="io", bufs=N_TILES) as iopool,
            tc.tile_pool(name="rem", bufs=1) as rpool,
        ):
            expd = cpool.tile([P, feat], mybir.dt.float32)
            # Host-replicated diagonal: plain full-rate load, first in the
            # SP queue so it lands before x tile 0.
            nc.sync.dma_start(expd[:], d)
            nc.scalar.activation(expd[:], expd[:], mybir.ActivationFunctionType.Exp)
            # DVE observer: absorbs the wait on the Exp so the muls below
            # carry exactly one wait (their own load DMA).
            scratch = cpool.tile([1, 1], mybir.dt.float32)
            nc.vector.tensor_copy(scratch[:], expd[0:1, 0:1])
            expd3 = expd[:].rearrange("p (o m) -> p o m", o=1)

            tiles = []
            for i in range(N_TILES):
                t = iopool.tile([P, fold * feat], mybir.dt.float32)
                t3 = t.rearrange("p (n m) -> p n m", n=fold)
                nc.sync.dma_start(t3, x_t[i])
                tiles.append(t3)
            tr = rpool.tile([REM, feat], mybir.dt.float32)
            tr3 = tr.rearrange("p (n m) -> p n m", n=1)
            nc.sync.dma_start(tr3, x_r)

            for i, t3 in enumerate(tiles):
                for j in range(fold):
                    nc.vector.tensor_mul(t3[:, j : j + 1], t3[:, j : j + 1], expd3)
                    nc.scalar.dma_start(o_t[i, :, j : j + 1], t3[:, j : j + 1])
            nc.vector.tensor_mul(tr3, tr3, expd3[0:REM])
            nc.scalar.dma_start(o_r, tr3)
    nc.finalize()
    return nc


def kernel(x, diagonal):
    from concourse.bass_utils import run_bass_kernel_spmd

    if "nc" not in _CACHE:
        _CACHE["nc"] = build_nc()
    nc = _CACHE["nc"]

    x = np.ascontiguousarray(x, dtype=np.float32)
    d = np.ascontiguousarray(
        np.broadcast_to(np.asarray(diagonal, dtype=np.float32), (P, FEAT))
    )
    in_maps = [{"x": x[c * ROWS : (c + 1) * ROWS], "d": d} for c in range(N_CORES)]
    res = run_bass_kernel_spmd(nc, in_maps, core_ids=list(range(N_CORES)))
    _CACHE["last_res"] = res
    return np.concatenate([r["out"] for r in res.results], axis=0)


# revision 11
# speedup vs baseline: 1.5236x; 1.5236x over previous
"""Trainium2 Bass kernel computing out = x * exp(diagonal).

x: (8192, 4096) float32, diagonal: (4096,) float32.
Data-parallel across 8 NeuronCores: each core handles 1024 rows of x;
the diagonal is replicated to every core.

Per-core program (pure streaming; the 16 SDMA engines aggregate
~435 GB/s and bound the kernel, so ~34 MiB of traffic floors at
~80 us; everything else must hide under that):

  * A stride-0 DRAM broadcast of the 16 KiB diagonal re-reads one HBM
    page 128x and is single-channel-bound (~25 us), so the host
    replicates the diagonal to (128, 4096) and the partition broadcast
    becomes a plain full-rate 2 MiB load. It rides the ACT HWDGE queue
    (stores start late, so that queue is idle early) while x loads own
    the SP queue from the first trigger.
  * exp runs in two column chunks so the first multiply only waits for
    the first 1024 columns; 1-element DVE copies observe each chunk so
    every multiply carries exactly one wait (its own load DMA).
  * The first 128 rows stream as two column-split tiles (0.5 MiB +
    1.5 MiB) so the first store enters the DMA queues at ~15 us; the
    remaining 896 rows stream as seven [128, 4096] tiles. All tiles
    are fresh buffers (no slot reuse => no WAR waits): HWDGE load on
    SP -> in-place DVE multiply -> HWDGE store on ACT.
"""

import numpy as np

BATCH, FEAT = 8192, 4096
N_CORES = 8
ROWS = BATCH // N_CORES   # 1024 rows per core
P = 128                   # SBUF partitions
C0 = 1024                 # first tile's column count

_CACHE = {}


def build_nc(rows=ROWS, feat=FEAT):
    import concourse.bacc as bacc
    import concourse.mybir as mybir
    from concourse import tile

    # Bacc (not plain Bass): its compile() pass splits multi-sem waits into
    # EventSemaphore chains -- TRN2 instructions carry at most one wait.
    nc = bacc.Bacc("TRN2", target_bir_lowering=False, debug=False)
    x = nc.dram_tensor("x", (rows, feat), mybir.dt.float32, kind="ExternalInput").ap()
    d = nc.dram_tensor("d", (P, feat), mybir.dt.float32, kind="ExternalInput").ap()
    out = nc.dram_tensor(
        "out", (rows, feat), mybir.dt.float32, kind="ExternalOutput"
    ).ap()

    n_big = rows // P - 1  # 7 full [128, feat] tiles for rows 128..1023
    x_t = x.rearrange("(s p) m -> s p m", p=P)
    o_t = out.rearrange("(s p) m -> s p m", p=P)

    with tile.TileContext(nc) as tc:
        with (
            tc.tile_pool(name="const", bufs=1) as cpool,
            tc.tile_pool(name="io", bufs=n_big) as iopool,
            tc.tile_pool(name="small", bufs=2) as spool,
        ):
            expd = cpool.tile([P, feat], mybir.dt.float32)
            # Host-replicated diagonal on the ACT queue (idle early).
            nc.scalar.dma_start(expd[:], d)
            # exp in two column chunks: the first 1024 columns unblock the
            # first multiply ~3 us earlier.
            nc.scalar.activation(
                expd[:, 0:C0], expd[:, 0:C0], mybir.ActivationFunctionType.Exp
            )
            nc.scalar.activation(
                expd[:, C0:feat], expd[:, C0:feat], mybir.ActivationFunctionType.Exp
            )

            # Column-split first row block: [128, 0:1024] then [128, 1024:].
            ta = spool.tile([P, C0], mybir.dt.float32)
            tb = spool.tile([P, feat - C0], mybir.dt.float32)
            nc.sync.dma_start(ta[:], x_t[0][:, 0:C0])
            nc.sync.dma_start(tb[:], x_t[0][:, C0:feat])
            tiles = []
            for i in range(n_big):
                t = iopool.tile([P, feat], mybir.dt.float32)
                nc.sync.dma_start(t[:], x_t[i + 1])
                tiles.append(t)

            # DVE observers absorb the waits on the two exp chunks so the
            # muls below carry exactly one wait (their own load DMA).
            scratch = cpool.tile([1, 2], mybir.dt.float32)
            nc.vector.tensor_copy(scratch[:, 0:1], expd[0:1, 0:1])
            nc.vector.tensor_mul(ta[:], ta[:], expd[:, 0:C0])
            nc.scalar.dma_start(o_t[0][:, 0:C0], ta[:])
            nc.vector.tensor_copy(scratch[:, 1:2], expd[0:1, feat - 1 : feat])
            nc.vector.tensor_mul(tb[:], tb[:], expd[:, C0:feat])
            nc.scalar.dma_start(o_t[0][:, C0:feat], tb[:])
            for i, t in enumerate(tiles):
                nc.vector.tensor_mul(t[:], t[:], expd[:])
                nc.scalar.dma_start(o_t[i + 1], t[:])
    nc.finalize()
    return nc


def kernel(x, diagonal):
    from concourse.bass_utils import run_bass_kernel_spmd

    if "nc" not in _CACHE:
        _CACHE["nc"] = build_nc()
    nc = _CACHE["nc"]

    x = np.ascontiguousarray(x, dtype=np.float32)
    d = np.ascontiguousarray(
        np.broadcast_to(np.asarray(diagonal, dtype=np.float32), (P, FEAT))
    )
    in_maps = [{"x": x[c * ROWS : (c + 1) * ROWS], "d": d} for c in range(N_CORES)]
    res = run_bass_kernel_spmd(nc, in_maps, core_ids=list(range(N_CORES)))
    _CACHE["last_res"] = res
    return np.concatenate([r["out"] for r in res.results], axis=0)


# revision 12
# speedup vs baseline: 1.8236x; 1.1969x over previous
"""Trainium2 Bass kernel computing out = x * exp(diagonal).

x: (8192, 4096) float32, diagonal: (4096,) float32.
Data-parallel across 8 NeuronCores: each core handles 1024 rows of x;
the 4096-float diagonal is replicated to every core.

Per-core program (pure streaming; the 16 SDMA engines aggregate
~435 GB/s and bound the kernel, so 32 MiB of x in+out traffic floors
at ~78 us; everything else must hide under that):

  1. diagonal loads as a [1, 4096] tile via one tiny HWDGE DMA issued
     on the SP queue BEFORE the x loads (per-engine rings are FIFO, so
     it completes first), ACT computes exp in place.
  2. Partition-broadcast WITHOUT any DMA: TensorE outer-product
     ones[1,128]^T @ expd[1,4096] -> PSUM [128, 4096] (8 matmuls, one
     per 512-float PSUM bank). Zero HBM/fabric bytes. A 1-element DVE
     copy observes the last matmul so the muls below carry exactly one
     wait (their own load DMA).
  3. x streams through 8 fresh [128, 4096] SBUF tiles (no slot reuse
     => no WAR waits): HWDGE load on SP -> in-place DVE multiply with
     operand b read from PSUM -> HWDGE store on ACT.
"""

import numpy as np

BATCH, FEAT = 8192, 4096
N_CORES = 8
ROWS = BATCH // N_CORES   # 1024 rows per core
P = 128                   # SBUF partitions
N_TILES = ROWS // P       # 8 tiles of [128, 4096] per core
PSUM_BANK = 512           # fp32 elems per PSUM bank (2 KiB)

_CACHE = {}


def build_nc(rows=ROWS, feat=FEAT):
    import concourse.bacc as bacc
    import concourse.mybir as mybir
    from concourse import tile

    # Bacc (not plain Bass): its compile() pass splits multi-sem waits into
    # EventSemaphore chains -- TRN2 instructions carry at most one wait.
    nc = bacc.Bacc("TRN2", target_bir_lowering=False, debug=False)
    x = nc.dram_tensor("x", (rows, feat), mybir.dt.float32, kind="ExternalInput").ap()
    d = nc.dram_tensor("d", (feat,), mybir.dt.float32, kind="ExternalInput").ap()
    out = nc.dram_tensor(
        "out", (rows, feat), mybir.dt.float32, kind="ExternalOutput"
    ).ap()

    n_tiles = rows // P
    x_t = x.rearrange("(s p) m -> s p m", p=P)
    o_t = out.rearrange("(s p) m -> s p m", p=P)
    d_row = d.rearrange("(r c) -> r c", r=1)

    with tile.TileContext(nc) as tc:
        with (
            tc.tile_pool(name="const", bufs=1) as cpool,
            tc.tile_pool(name="psum", bufs=1, space="PSUM") as ppool,
            tc.tile_pool(name="io", bufs=n_tiles) as iopool,
        ):
            d1 = cpool.tile([1, feat], mybir.dt.float32)
            ones = cpool.tile([1, P], mybir.dt.float32)
            expd = ppool.tile([P, feat], mybir.dt.float32)

            # diagonal -> [1, feat]: first DMA on the SP queue, so its
            # descriptors drain before x tile 0's on every engine ring.
            nc.sync.dma_start(d1[:], d_row)
            nc.scalar.activation(d1[:], d1[:], mybir.ActivationFunctionType.Exp)
            nc.vector.memset(ones[:], 1.0)
            # Broadcast across partitions: ones^T @ expd, one matmul per
            # PSUM bank (512 fp32).
            for b in range(feat // PSUM_BANK):
                sl = slice(b * PSUM_BANK, (b + 1) * PSUM_BANK)
                nc.tensor.matmul(
                    expd[:, sl], ones[:], d1[:, sl], start=True, stop=True
                )
            # DVE observer: absorbs the wait on the matmuls so the muls
            # below carry exactly one wait (their own load DMA).
            scratch = cpool.tile([1, 1], mybir.dt.float32)
            nc.vector.tensor_copy(scratch[:], expd[0:1, 0:1])

            tiles = []
            for i in range(n_tiles):
                t = iopool.tile([P, feat], mybir.dt.float32)
                nc.sync.dma_start(t[:], x_t[i])
                tiles.append(t)
            for i, t in enumerate(tiles):
                nc.vector.tensor_mul(t[:], t[:], expd[:])
                nc.scalar.dma_start(o_t[i], t[:])
    nc.finalize()
    return nc


def kernel(x, diagonal):
    from concourse.bass_utils import run_bass_kernel_spmd

    if "nc" not in _CACHE:
        _CACHE["nc"] = build_nc()
    nc = _CACHE["nc"]

    x = np.ascontiguousarray(x, dtype=np.float32)
    d = np.ascontiguousarray(diagonal, dtype=np.float32)
    in_maps = [{"x": x[c * ROWS : (c + 1) * ROWS], "d": d} for c in range(N_CORES)]
    res = run_bass_kernel_spmd(nc, in_maps, core_ids=list(range(N_CORES)))
    _CACHE["last_res"] = res
    return np.concatenate([r["out"] for r in res.results], axis=0)
